# revision 9
# baseline (speedup 1.0000x reference)
"""Trainium2 Bass kernel for nn_CompositionBlock (gnn_message_passing).

Reference semantics (per batch b, S=2048 tokens, T=128 dims):
    h        = tanh(token)                               # [S, T]
    val[s,t] = sum_pq token[s,p] W[t,p,q] h[s,q] + b_comp[t]
    act      = tanh(val)
    delta    = w_red[s] * (act[s,t] - tanh(b_comp)[t])
    out[i,t] = sum_s w_red[s]*tanh(b_comp)[t] + b_red
               + sum_{s: heads[s]==i} delta[s,t]

Sharding: data-parallel over batch B=8 -> one batch per NeuronCore; W and
the small vectors are replicated. No collectives needed.

Device algorithm per core:
  MM1   (PE):  A[s, (t,q)] = tokenT_tile.T @ W[p,(t,q)] into PSUM, fp16 in.
  TTR   (VE):  val[s,t] = sum_q A[s,(t,q)] * h[s,q]  (+ b_comp[t] seed).
  ACT:         act = tanh(val);  delta = w*(act - tanh(b_comp))  (VE).
  one-hot (VE): MT[j,i] = (heads[j] == i) via is_equal against an iota row.
  MM2   (PE):  outT[t,i] += delta_j.T @ MT_j  accumulated over j-tiles.
  epilogue:    outT += base[t] (per-partition scalar), DMA out, host transpose.
"""

import numpy as np

import concourse.bass as bass
import concourse.tile as tile
from concourse import bacc, mybir
from concourse.bass_utils import run_bass_kernel_spmd

B, S, T = 8, 2048, 128
P = 128
N_CORES = 8
NST = S // P  # 16 s-tiles per batch
F32 = mybir.dt.float32
F16 = mybir.dt.float16
I32 = mybir.dt.int32
AF = mybir.ActivationFunctionType
ALU = mybir.AluOpType

_NC_CACHE = {}

import os
_ABLATE = set(os.environ.get("KERNEL_ABLATE", "").split(","))


def build_nc():
    nc = bacc.Bacc("TRN2", target_bir_lowering=False, debug=False,
                   num_devices=N_CORES)

    # DRAM parameters (per-core shapes; host preps layouts).
    tokT_d = nc.dram_tensor("tokT", [T, S], F32, kind="ExternalInput").ap()
    tok_d = nc.dram_tensor("tok", [P, S], F32, kind="ExternalInput").ap()
    w_ptq_d = nc.dram_tensor("w_ptq", [P, T * T], F32, kind="ExternalInput").ap()
    bcomp_d = nc.dram_tensor("bcomp", [1, T], F32, kind="ExternalInput").ap()
    bcompT_d = nc.dram_tensor("bcompT", [T, 1], F32, kind="ExternalInput").ap()
    wred_d = nc.dram_tensor("wred", [P, NST], F32, kind="ExternalInput").ap()
    heads_d = nc.dram_tensor("heads", [P, NST], I32, kind="ExternalInput").ap()
    bred_d = nc.dram_tensor("bred", [1, 1], F32, kind="ExternalInput").ap()
    iota_d = nc.dram_tensor("iota", [1, S], F16, kind="ExternalInput").ap()
    outT_d = nc.dram_tensor("outT", [T, S], F32, kind="ExternalOutput").ap()

    with tile.TileContext(nc) as tc:
        _body(tc, tokT_d, tok_d, w_ptq_d, bcomp_d, bcompT_d, wred_d, heads_d,
              bred_d, iota_d, outT_d)
    nc.compile()
    return nc


def _body(tc, tokT_d, tok_d, w_ptq_d, bcomp_d, bcompT_d, wred_d, heads_d,
          bred_d, iota_d, outT_d):
    nc = tc.nc
    from contextlib import ExitStack
    with ExitStack() as ctx:
        const = ctx.enter_context(tc.tile_pool(name="const", bufs=1))
        hpool = ctx.enter_context(tc.tile_pool(name="hpool", bufs=3))
        vpool = ctx.enter_context(tc.tile_pool(name="vpool", bufs=3))
        spool = ctx.enter_context(tc.tile_pool(name="spool", bufs=3))
        mtpool = ctx.enter_context(tc.tile_pool(name="mtpool", bufs=2))
        junk = ctx.enter_context(tc.tile_pool(name="junk", bufs=4))
        psumA = ctx.enter_context(tc.tile_pool(name="psumA", bufs=3, space="PSUM"))
        psumO = ctx.enter_context(tc.tile_pool(name="psumO", bufs=1, space="PSUM"))
        psumS = ctx.enter_context(tc.tile_pool(name="psumS", bufs=1, space="PSUM"))

        # ---- constants / weights ----
        w_sb = const.tile([P, T * T], F16)
        nc.gpsimd.dma_start(out=w_sb[:], in_=w_ptq_d[:])  # f32 -> fp16 cast
        tokT_sb = const.tile([P, S], F16)
        nc.gpsimd.dma_start(out=tokT_sb[:], in_=tokT_d[:])
        tok_sb = const.tile([P, S], F32)
        nc.sync.dma_start(out=tok_sb[:], in_=tok_d[:])
        iota_sb = const.tile([P, S], F16)
        nc.sync.dma_start(out=iota_sb[:], in_=iota_d[0:1, :].to_broadcast((P, S)))
        bcompR = const.tile([P, T], F32)
        nc.sync.dma_start(out=bcompR[:], in_=bcomp_d[0:1, :].to_broadcast((P, T)))
        basevR = const.tile([P, T], F32)
        nc.scalar.activation(basevR[:], bcompR[:], AF.Tanh)
        wred_sb = const.tile([P, NST], F32)
        nc.sync.dma_start(out=wred_sb[:], in_=wred_d[:])
        heads_sb = const.tile([P, NST], I32)
        nc.sync.dma_start(out=heads_sb[:], in_=heads_d[:])
        headsF = const.tile([P, NST], F16)
        nc.vector.tensor_copy(headsF[:], heads_sb[:])
        bcompT_sb = const.tile([T, 1], F32)
        nc.sync.dma_start(out=bcompT_sb[:], in_=bcompT_d[:])
        basevT = const.tile([T, 1], F32)
        nc.scalar.activation(basevT[:], bcompT_sb[:], AF.Tanh)
        bredR = const.tile([P, 1], F32)
        nc.sync.dma_start(out=bredR[:], in_=bred_d[0:1, 0:1].to_broadcast((P, 1)))

        # ---- Sw = sum(w_red); baseT[t] = Sw*tanh(b_comp[t]) + b_red ----
        baseT = const.tile([P, 1], F32)
        if "nosw" in _ABLATE:
            nc.gpsimd.memset(baseT[:], 0.0)
        else:
            wsum_p = const.tile([P, 1], F32)
            nc.vector.tensor_reduce(out=wsum_p[:], in_=wred_sb[:], op=ALU.add,
                                    axis=mybir.AxisListType.X)
            ones_p = const.tile([P, 1], F32)
            nc.gpsimd.memset(ones_p[:], 1.0)
            sw_ps = psumS.tile([1, 1], F32, space="PSUM")
            nc.tensor.matmul(sw_ps[:], lhsT=wsum_p[:], rhs=ones_p[:],
                             start=True, stop=True)
            sw_sb = const.tile([1, 1], F32)
            nc.vector.tensor_copy(sw_sb[:], sw_ps[:])
            swR = const.tile([P, 1], F32)
            nc.gpsimd.partition_broadcast(swR[:], sw_sb[0:1, :])
            nc.vector.scalar_tensor_tensor(out=baseT[:], in0=basevT[:],
                                           scalar=swR[:], in1=bredR[:],
                                           op0=ALU.mult, op1=ALU.add)

        # ---- persistent output accumulators: outT[t, i] in PSUM ----
        OTs = [psumO.tile([P, 512], F32, space="PSUM", tag=f"OT{c}",
                          name=f"OT{c}") for c in range(4)]

        # ---- main loop over s-tiles ----
        for i in range(NST):
            sl = slice(P * i, P * (i + 1))
            h_i = hpool.tile([P, T], F32)
            nc.scalar.activation(h_i[:], tok_sb[:, sl], AF.Tanh)
            val_i = vpool.tile([P, T], F32)
            for tg in range(T // 4):
                A = psumA.tile([P, 512], F32, space="PSUM", tag="A")
                nc.tensor.matmul(A[:], lhsT=tokT_sb[:, sl],
                                 rhs=w_sb[:, 512 * tg: 512 * (tg + 1)],
                                 start=True, stop=True)
                if "nottr" in _ABLATE:
                    if tg == 0:
                        nc.vector.tensor_scalar_mul(val_i[:], A[:, 0:T], 0.001)
                    continue
                for k in range(4):
                    t = 4 * tg + k
                    jt = junk.tile([P, T], F16, tag="junk")
                    nc.vector.scalar_tensor_tensor(
                        out=jt[:], in0=A[:, T * k: T * (k + 1)], scalar=1.0,
                        in1=h_i[:], op0=ALU.mult, op1=ALU.mult,
                        accum_out=val_i[:, t: t + 1])
            valb_i = vpool.tile([P, T], F32, tag="valb")
            nc.vector.tensor_add(valb_i[:], val_i[:], bcompR[:])
            act_i = spool.tile([P, T], F16, tag="act")
            nc.scalar.activation(act_i[:], valb_i[:], AF.Tanh)
            wbv_i = spool.tile([P, T], F16, tag="wbv")
            nc.vector.tensor_scalar_mul(wbv_i[:], basevR[:],
                                        wred_sb[:, i: i + 1])
            delta_i = spool.tile([P, T], F16, tag="delta")
            nc.vector.scalar_tensor_tensor(out=delta_i[:], in0=act_i[:],
                                           scalar=wred_sb[:, i: i + 1],
                                           in1=wbv_i[:],
                                           op0=ALU.mult, op1=ALU.subtract)
            mt_i = mtpool.tile([P, S], F16, tag="mt")
            nc.vector.tensor_tensor(out=mt_i[:],
                                    in0=headsF[:, i: i + 1].to_broadcast((P, S)),
                                    in1=iota_sb[:], op=ALU.is_equal)
            if "noscat" in _ABLATE:
                continue
            for c in range(4):
                nc.tensor.matmul(OTs[c][:], lhsT=delta_i[:],
                                 rhs=mt_i[:, 512 * c: 512 * (c + 1)],
                                 start=(i == 0), stop=(i == NST - 1))

        # ---- epilogue: outT = OT + baseT[t]; store ----
        outT_sb = const.tile([P, S], F32)
        if "noscat" in _ABLATE:
            for c in range(4):
                nc.vector.tensor_scalar_add(
                    outT_sb[:, 512 * c: 512 * (c + 1)],
                    OTs[c][:], baseT[:])
        else:
            for c in range(4):
                nc.vector.tensor_scalar_add(
                    outT_sb[:, 512 * c: 512 * (c + 1)],
                    OTs[c][:], baseT[:])
        nc.sync.dma_start(out=outT_d[:], in_=outT_sb[:])


def _prep_inputs(token_embeddings, dep_heads, W_comp, b_comp, w_red, b_red):
    """Host-side sharding + layout prep. One in_map per core (= per batch)."""
    token = np.ascontiguousarray(np.asarray(token_embeddings, np.float32))
    heads = np.asarray(dep_heads, np.int32)
    W = np.ascontiguousarray(np.asarray(W_comp, np.float32))
    w_ptq = np.ascontiguousarray(W.transpose(1, 0, 2).reshape(P, T * T))
    bcomp = np.asarray(b_comp, np.float32).reshape(1, T)
    bcompT = np.ascontiguousarray(bcomp.reshape(T, 1))
    wred = np.ascontiguousarray(
        np.asarray(w_red, np.float32).reshape(NST, P).T)
    bred = np.asarray(b_red, np.float32).reshape(1, 1)
    iota = np.arange(S, dtype=np.float16).reshape(1, S)

    in_maps = []
    for b in range(B):
        tok_b = token[b]  # [S, T]
        in_maps.append({
            "tokT": np.ascontiguousarray(tok_b.T),
            "tok": np.ascontiguousarray(
                tok_b.reshape(NST, P, T).transpose(1, 0, 2).reshape(P, S)),
            "w_ptq": w_ptq,
            "bcomp": bcomp,
            "bcompT": bcompT,
            "wred": wred,
            "heads": np.ascontiguousarray(heads[b].reshape(NST, P).T),
            "bred": bred,
            "iota": iota,
        })
    return in_maps


def kernel(**inputs):
    if "nc" not in _NC_CACHE:
        _NC_CACHE["nc"] = build_nc()
    nc = _NC_CACHE["nc"]
    in_maps = _prep_inputs(
        inputs["token_embeddings"], inputs["dep_heads"], inputs["W_comp"],
        inputs["b_comp"], inputs["w_red"], inputs["b_red"])
    res = run_bass_kernel_spmd(nc, in_maps, core_ids=list(range(N_CORES)))
    out = np.empty((B, S, T), np.float32)
    for b in range(B):
        out[b] = res.results[b]["outT"].T
    return out


# revision 17
# speedup vs baseline: 1.8412x; 1.8412x over previous
"""Trainium2 Bass kernel for nn_CompositionBlock (gnn_message_passing).

Reference semantics (per batch b, S=2048 tokens, T=128 dims):
    h        = tanh(token)                               # [S, T]
    val[s,t] = sum_pq token[s,p] W[t,p,q] h[s,q] + b_comp[t]
    act      = tanh(val)
    delta    = w_red[s] * (act[s,t] - tanh(b_comp)[t])
    out[i,t] = sum_s w_red[s]*tanh(b_comp)[t] + b_red
               + sum_{s: heads[s]==i} delta[s,t]

Sharding: data-parallel over batch B=8 -> one batch per NeuronCore; W and
the small vectors replicated. No collectives.

Device algorithm per core (all matmuls fp16 in / f32 psum accum):
  MM1 (PE):  A_t[q, s] = W_t[p,q].T @ tokenT[p, s]   (per t, s-group of 512)
  TT  (VE):  Z_t[q, s] = A_t * hT[q, s]              (the only big VE pass)
  MM2 (PE):  valT[t, s] += E_t.T @ Z_t  where E_t = staircase slice with a
             ones column at position t -> accumulates sum_q Z_t into row t.
  ACT:       actT = tanh(valT + b_comp[t])  (per-partition bias)
  deltaT = actT - tanh(b_comp)[t];  DMA-xbar transpose -> delta[j, t];
  delta_w = w_red[j] * delta.
  one-hot (GPSIMD): MT[j,i] = (heads[j] == i) via is_equal vs iota row.
  MM3 (PE):  outT[t,i] += delta_w_j.T @ MT_j over j-tiles; += base[t]; DMA.
Host transposes outT -> out per batch at gather time.
"""

import os
from contextlib import ExitStack

import numpy as np

import concourse.bass as bass
import concourse.tile as tile
from concourse import bacc, mybir
from concourse.bass_utils import run_bass_kernel_spmd

B, S, T = 8, 2048, 128
P = 128
N_CORES = 8
NST = S // P      # 16 s-tiles of 128
NSG = S // 512    # 4 s-groups of 512
F32 = mybir.dt.float32
F16 = mybir.dt.float16
I32 = mybir.dt.int32
AF = mybir.ActivationFunctionType
ALU = mybir.AluOpType

_NC_CACHE = {}


def build_nc():
    nc = bacc.Bacc("TRN2", target_bir_lowering=False, debug=False,
                   num_devices=N_CORES)

    tokT_d = nc.dram_tensor("tokT", [T, S], F32, kind="ExternalInput").ap()
    w_ptq_d = nc.dram_tensor("w_ptq", [P, T * T], F32, kind="ExternalInput").ap()
    bcompT_d = nc.dram_tensor("bcompT", [T, 1], F32, kind="ExternalInput").ap()
    wred_d = nc.dram_tensor("wred", [P, NST], F32, kind="ExternalInput").ap()
    heads_d = nc.dram_tensor("heads", [P, NST], I32, kind="ExternalInput").ap()
    bred_d = nc.dram_tensor("bred", [1, 1], F32, kind="ExternalInput").ap()
    iota_d = nc.dram_tensor("iota", [1, S], F16, kind="ExternalInput").ap()
    outT_d = nc.dram_tensor("outT", [T, S], F32, kind="ExternalOutput").ap()

    with tile.TileContext(nc) as tc:
        _body(tc, tokT_d, w_ptq_d, bcompT_d, wred_d, heads_d, bred_d, iota_d,
              outT_d)
    nc.compile()
    return nc


def _body(tc, tokT_d, w_ptq_d, bcompT_d, wred_d, heads_d, bred_d, iota_d,
          outT_d):
    nc = tc.nc
    with ExitStack() as ctx:
        const = ctx.enter_context(tc.tile_pool(name="const", bufs=1))
        zpool = ctx.enter_context(tc.tile_pool(name="zpool", bufs=4))
        spool = ctx.enter_context(tc.tile_pool(name="spool", bufs=2))
        djp = ctx.enter_context(tc.tile_pool(name="djp", bufs=3))
        dwp = ctx.enter_context(tc.tile_pool(name="dwp", bufs=1))
        mtp = ctx.enter_context(tc.tile_pool(name="mtp", bufs=1))


        # ---- constants / inputs ----
        w_sb = const.tile([P, T * T], F16)
        nc.gpsimd.dma_start(out=w_sb[:], in_=w_ptq_d[:])  # cast f32->fp16
        tokT_sb = const.tile([P, S], F16)
        nc.gpsimd.dma_start(out=tokT_sb[:], in_=tokT_d[:])
        hT_sb = const.tile([P, S], F16)
        nc.scalar.activation(hT_sb[:], tokT_sb[:], AF.Tanh)
        iota_sb = const.tile([P, S], F16)
        nc.sync.dma_start(out=iota_sb[:], in_=iota_d[0:1, :].to_broadcast((P, S)))
        wred_sb = const.tile([P, NST], F32)
        nc.sync.dma_start(out=wred_sb[:], in_=wred_d[:])
        heads_sb = const.tile([P, NST], I32)
        nc.sync.dma_start(out=heads_sb[:], in_=heads_d[:])
        headsF = const.tile([P, NST], F32)
        nc.vector.tensor_copy(headsF[:], heads_sb[:])
        bcompT_sb = const.tile([T, 1], F32)
        nc.sync.dma_start(out=bcompT_sb[:], in_=bcompT_d[:])
        basevT = const.tile([T, 1], F32)
        nc.scalar.activation(basevT[:], bcompT_sb[:], AF.Tanh)
        bredR = const.tile([P, 1], F32)
        nc.sync.dma_start(out=bredR[:], in_=bred_d[0:1, 0:1].to_broadcast((P, 1)))
        # staircase: Q[:, P-1] = 1, else 0; E_t = Q[:, P-1-t : 2P-1-t]
        Q = const.tile([P, 2 * P - 1], F16)
        nc.gpsimd.memset(Q[:], 0.0)
        nc.gpsimd.memset(Q[:, P - 1: P], 1.0)

        # ---- Sw = sum(w_red); baseT[t] = Sw*tanh(b_comp)[t] + b_red ----
        wsum_p = const.tile([P, 1], F32)
        nc.vector.tensor_reduce(out=wsum_p[:], in_=wred_sb[:], op=ALU.add,
                                axis=mybir.AxisListType.X)
        from concourse import bass_isa
        swR = const.tile([P, 1], F32)
        nc.gpsimd.partition_all_reduce(swR[:], wsum_p[:], channels=P,
                                       reduce_op=bass_isa.ReduceOp.add)
        baseT = const.tile([P, 1], F32)
        nc.vector.scalar_tensor_tensor(out=baseT[:], in0=basevT[:],
                                       scalar=swR[:], in1=bredR[:],
                                       op0=ALU.mult, op1=ALU.add)

        # ---- one-hot scatter matrices: MT[j, i] = (iota[i] == head[j]) ----
        mts = []
        for j in range(NST):
            mt_j = mtp.tile([P, S], F16, tag=f"mt{j}", name=f"mt{j}")
            nc.vector.tensor_scalar(out=mt_j[:], in0=iota_sb[:],
                                    scalar1=headsF[:, j: j + 1], scalar2=None,
                                    op0=ALU.is_equal)
            mts.append(mt_j)

        # ---- main loop: s-groups of 512 ----
        dws = []
        with tc.tile_pool(name="psumA", bufs=6, space="PSUM") as psumA, \
             tc.tile_pool(name="psumV", bufs=2, space="PSUM") as psumV:
            for g in range(NSG):
                gs = slice(512 * g, 512 * (g + 1))
                V = psumV.tile([P, 512], F32, space="PSUM", tag="V", name="V")
                for t in range(T):
                    A = psumA.tile([P, 512], F32, space="PSUM", tag="A",
                                   name="A")
                    nc.tensor.matmul(A[:], lhsT=w_sb[:, T * t: T * (t + 1)],
                                     rhs=tokT_sb[:, gs], start=True, stop=True)
                    Z = zpool.tile([P, 512], F16, tag="Z", name="Z")
                    nc.vector.tensor_tensor(out=Z[:], in0=A[:],
                                            in1=hT_sb[:, gs], op=ALU.mult)
                    nc.tensor.matmul(V[:], lhsT=Q[:, P - 1 - t: 2 * P - 1 - t],
                                     rhs=Z[:], start=(t == 0),
                                     stop=(t == T - 1))
                actT = spool.tile([P, 512], F16, tag="actT", name="actT")
                nc.scalar.activation(actT[:], V[:], AF.Tanh, bias=bcompT_sb[:])
                dT = spool.tile([P, 512], F16, tag="dT", name="dT")
                nc.vector.tensor_scalar_sub(dT[:], actT[:], basevT[:])
                for k in range(4):
                    j = 4 * g + k
                    dj = djp.tile([P, P], F16, tag="dj", name="dj")
                    nc.sync.dma_start_transpose(out=dj[:],
                                                in_=dT[:, P * k: P * (k + 1)])
                    dw_j = dwp.tile([P, P], F16, tag=f"dw{j}", name=f"dw{j}")
                    nc.vector.tensor_scalar_mul(dw_j[:], dj[:],
                                                wred_sb[:, j: j + 1])
                    dws.append(dw_j)

        # ---- scatter: outT[t, i] = sum_j delta_w[j, t] * MT[j, i] + base ----
        outT_sb = const.tile([P, S], F32)
        with tc.tile_pool(name="psumO", bufs=1, space="PSUM") as psumO:
            for c in range(4):
                OT = psumO.tile([P, 512], F32, space="PSUM", tag=f"OT{c}",
                                name=f"OT{c}")
                for j in range(NST):
                    nc.tensor.matmul(OT[:], lhsT=dws[j][:],
                                     rhs=mts[j][:, 512 * c: 512 * (c + 1)],
                                     start=(j == 0), stop=(j == NST - 1))
                nc.vector.tensor_scalar_add(
                    outT_sb[:, 512 * c: 512 * (c + 1)], OT[:], baseT[:])
        nc.sync.dma_start(out=outT_d[:], in_=outT_sb[:])


def _prep_inputs(token_embeddings, dep_heads, W_comp, b_comp, w_red, b_red):
    """Host-side sharding + layout prep. One in_map per core (= per batch)."""
    token = np.ascontiguousarray(np.asarray(token_embeddings, np.float32))
    heads = np.asarray(dep_heads, np.int32)
    W = np.ascontiguousarray(np.asarray(W_comp, np.float32))
    w_ptq = np.ascontiguousarray(W.transpose(1, 0, 2).reshape(P, T * T))
    bcompT = np.ascontiguousarray(
        np.asarray(b_comp, np.float32).reshape(T, 1))
    wred = np.ascontiguousarray(
        np.asarray(w_red, np.float32).reshape(NST, P).T)
    bred = np.asarray(b_red, np.float32).reshape(1, 1)
    iota = np.arange(S, dtype=np.float16).reshape(1, S)

    in_maps = []
    for b in range(B):
        in_maps.append({
            "tokT": np.ascontiguousarray(token[b].T),
            "w_ptq": w_ptq,
            "bcompT": bcompT,
            "wred": wred,
            "heads": np.ascontiguousarray(heads[b].reshape(NST, P).T),
            "bred": bred,
            "iota": iota,
        })
    return in_maps


def kernel(**inputs):
    if "nc" not in _NC_CACHE:
        _NC_CACHE["nc"] = build_nc()
    nc = _NC_CACHE["nc"]
    in_maps = _prep_inputs(
        inputs["token_embeddings"], inputs["dep_heads"], inputs["W_comp"],
        inputs["b_comp"], inputs["w_red"], inputs["b_red"])
    res = run_bass_kernel_spmd(nc, in_maps, core_ids=list(range(N_CORES)))
    out = np.empty((B, S, T), np.float32)
    for b in range(B):
        out[b] = res.results[b]["outT"].T
    return out


# revision 19
# speedup vs baseline: 2.3218x; 1.2611x over previous
"""Trainium2 Bass kernel for nn_CompositionBlock (gnn_message_passing).

Reference semantics (per batch b, S=2048 tokens, T=128 dims):
    h        = tanh(token)                               # [S, T]
    val[s,t] = sum_pq token[s,p] W[t,p,q] h[s,q] + b_comp[t]
    act      = tanh(val)
    delta    = w_red[s] * (act[s,t] - tanh(b_comp)[t])
    out[i,t] = sum_s w_red[s]*tanh(b_comp)[t] + b_red
               + sum_{s: heads[s]==i} delta[s,t]

Sharding: data-parallel over batch B=8 -> one batch per NeuronCore; W and
the small vectors replicated. No collectives.

Device algorithm per core (all matmuls fp16 in / f32 psum accum):
  MM1 (PE):  A_t[q, s] = W_t[p,q].T @ tokenT[p, s]   (per t, s-group of 512)
  TT  (VE):  Z_t[q, s] = A_t * hT[q, s]              (the only big VE pass)
  MM2 (PE):  valT[t, s] += E_t.T @ Z_t  where E_t = staircase slice with a
             ones column at position t -> accumulates sum_q Z_t into row t.
  ACT:       actT = tanh(valT + b_comp[t])  (per-partition bias)
  deltaT = actT - tanh(b_comp)[t];  DMA-xbar transpose -> delta[j, t];
  delta_w = w_red[j] * delta.
  one-hot (GPSIMD): MT[j,i] = (heads[j] == i) via is_equal vs iota row.
  MM3 (PE):  outT[t,i] += delta_w_j.T @ MT_j over j-tiles; += base[t]; DMA.
Host transposes outT -> out per batch at gather time.
"""

import os
from contextlib import ExitStack

import numpy as np

import concourse.bass as bass
import concourse.tile as tile
from concourse import bacc, mybir
from concourse.bass_utils import run_bass_kernel_spmd

B, S, T = 8, 2048, 128
P = 128
N_CORES = 8
NST = S // P      # 16 s-tiles of 128
NSG = S // 512    # 4 s-groups of 512
F32 = mybir.dt.float32
F16 = mybir.dt.float16
I32 = mybir.dt.int32
AF = mybir.ActivationFunctionType
ALU = mybir.AluOpType

_NC_CACHE = {}


def build_nc():
    nc = bacc.Bacc("TRN2", target_bir_lowering=False, debug=False,
                   num_devices=N_CORES)

    tokT_d = nc.dram_tensor("tokT", [T, S], F32, kind="ExternalInput").ap()
    w_ptq_d = nc.dram_tensor("w_ptq", [P, T * T], F32, kind="ExternalInput").ap()
    bcompT_d = nc.dram_tensor("bcompT", [T, 1], F32, kind="ExternalInput").ap()
    wred_d = nc.dram_tensor("wred", [P, NST], F32, kind="ExternalInput").ap()
    heads_d = nc.dram_tensor("heads", [P, NST], I32, kind="ExternalInput").ap()
    bred_d = nc.dram_tensor("bred", [1, 1], F32, kind="ExternalInput").ap()
    iota_d = nc.dram_tensor("iota", [1, S], F16, kind="ExternalInput").ap()
    outT_d = nc.dram_tensor("outT", [T, S], F32, kind="ExternalOutput").ap()

    with tile.TileContext(nc) as tc:
        _body(tc, tokT_d, w_ptq_d, bcompT_d, wred_d, heads_d, bred_d, iota_d,
              outT_d)
    nc.compile()
    return nc


def _body(tc, tokT_d, w_ptq_d, bcompT_d, wred_d, heads_d, bred_d, iota_d,
          outT_d):
    nc = tc.nc
    with ExitStack() as ctx:
        const = ctx.enter_context(tc.tile_pool(name="const", bufs=1))
        zpool = ctx.enter_context(tc.tile_pool(name="zpool", bufs=6))
        a16p = ctx.enter_context(tc.tile_pool(name="a16p", bufs=4))
        spool = ctx.enter_context(tc.tile_pool(name="spool", bufs=2))
        djp = ctx.enter_context(tc.tile_pool(name="djp", bufs=3))
        dwp = ctx.enter_context(tc.tile_pool(name="dwp", bufs=1))
        mtp = ctx.enter_context(tc.tile_pool(name="mtp", bufs=1))


        # ---- constants / inputs ----
        w_sb = const.tile([P, T * T], F16)
        nc.gpsimd.dma_start(out=w_sb[:], in_=w_ptq_d[:])  # cast f32->fp16
        tokT_sb = const.tile([P, S], F16)
        nc.gpsimd.dma_start(out=tokT_sb[:], in_=tokT_d[:])
        hT_sb = const.tile([P, S], F16)
        nc.scalar.activation(hT_sb[:], tokT_sb[:], AF.Tanh)
        iota_sb = const.tile([P, S], F16)
        nc.sync.dma_start(out=iota_sb[:], in_=iota_d[0:1, :].to_broadcast((P, S)))
        wred_sb = const.tile([P, NST], F32)
        nc.sync.dma_start(out=wred_sb[:], in_=wred_d[:])
        heads_sb = const.tile([P, NST], I32)
        nc.sync.dma_start(out=heads_sb[:], in_=heads_d[:])
        headsF = const.tile([P, NST], F32)
        nc.vector.tensor_copy(headsF[:], heads_sb[:])
        bcompT_sb = const.tile([T, 1], F32)
        nc.sync.dma_start(out=bcompT_sb[:], in_=bcompT_d[:])
        basevT = const.tile([T, 1], F32)
        nc.scalar.activation(basevT[:], bcompT_sb[:], AF.Tanh)
        bredR = const.tile([P, 1], F32)
        nc.sync.dma_start(out=bredR[:], in_=bred_d[0:1, 0:1].to_broadcast((P, 1)))
        # staircase: Q[:, P-1] = 1, else 0; E_t = Q[:, P-1-t : 2P-1-t]
        Q = const.tile([P, 2 * P - 1], F16)
        nc.gpsimd.memset(Q[:], 0.0)
        nc.gpsimd.memset(Q[:, P - 1: P], 1.0)

        # ---- Sw = sum(w_red); baseT[t] = Sw*tanh(b_comp)[t] + b_red ----
        wsum_p = const.tile([P, 1], F32)
        nc.vector.tensor_reduce(out=wsum_p[:], in_=wred_sb[:], op=ALU.add,
                                axis=mybir.AxisListType.X)
        from concourse import bass_isa
        swR = const.tile([P, 1], F32)
        nc.gpsimd.partition_all_reduce(swR[:], wsum_p[:], channels=P,
                                       reduce_op=bass_isa.ReduceOp.add)
        baseT = const.tile([P, 1], F32)
        nc.vector.scalar_tensor_tensor(out=baseT[:], in0=basevT[:],
                                       scalar=swR[:], in1=bredR[:],
                                       op0=ALU.mult, op1=ALU.add)

        # ---- one-hot scatter matrices: MT[j, i] = (iota[i] == head[j]) ----
        mts = []
        for j in range(NST):
            mt_j = mtp.tile([P, S], F16, tag=f"mt{j}", name=f"mt{j}")
            nc.vector.tensor_scalar(out=mt_j[:], in0=iota_sb[:],
                                    scalar1=headsF[:, j: j + 1], scalar2=None,
                                    op0=ALU.is_equal)
            mts.append(mt_j)

        # ---- main loop: s-groups of 512 ----
        dws = []
        with tc.tile_pool(name="psumA", bufs=6, space="PSUM") as psumA, \
             tc.tile_pool(name="psumV", bufs=2, space="PSUM") as psumV:
            for g in range(NSG):
                gs = slice(512 * g, 512 * (g + 1))
                V = psumV.tile([P, 512], F32, space="PSUM", tag="V", name="V")
                for t in range(T):
                    A = psumA.tile([P, 512], F32, space="PSUM", tag="A",
                                   name="A")
                    nc.tensor.matmul(A[:], lhsT=w_sb[:, T * t: T * (t + 1)],
                                     rhs=tokT_sb[:, gs], start=True, stop=True)
                    Z = zpool.tile([P, 512], F16, tag="Z", name="Z")
                    if t % 4 != 3:
                        # ScalarE downcast to fp16 SBUF so the DVE multiply
                        # runs in 2x_1p packed mode; ~3/4 of tiles balances
                        # ACT and DVE busy time.
                        A16 = a16p.tile([P, 512], F16, tag="A16", name="A16")
                        nc.scalar.activation(A16[:], A[:], AF.Copy)
                        nc.vector.tensor_tensor(out=Z[:], in0=A16[:],
                                                in1=hT_sb[:, gs], op=ALU.mult)
                    else:
                        nc.vector.tensor_tensor(out=Z[:], in0=A[:],
                                                in1=hT_sb[:, gs], op=ALU.mult)
                    nc.tensor.matmul(V[:], lhsT=Q[:, P - 1 - t: 2 * P - 1 - t],
                                     rhs=Z[:], start=(t == 0),
                                     stop=(t == T - 1))
                actT = spool.tile([P, 512], F16, tag="actT", name="actT")
                nc.scalar.activation(actT[:], V[:], AF.Tanh, bias=bcompT_sb[:])
                dT = spool.tile([P, 512], F16, tag="dT", name="dT")
                nc.vector.tensor_scalar_sub(dT[:], actT[:], basevT[:])
                for k in range(4):
                    j = 4 * g + k
                    dj = djp.tile([P, P], F16, tag="dj", name="dj")
                    nc.sync.dma_start_transpose(out=dj[:],
                                                in_=dT[:, P * k: P * (k + 1)])
                    dw_j = dwp.tile([P, P], F16, tag=f"dw{j}", name=f"dw{j}")
                    nc.vector.tensor_scalar_mul(dw_j[:], dj[:],
                                                wred_sb[:, j: j + 1])
                    dws.append(dw_j)

        # ---- scatter: outT[t, i] = sum_j delta_w[j, t] * MT[j, i] + base ----
        outT_sb = const.tile([P, S], F32)
        with tc.tile_pool(name="psumO", bufs=1, space="PSUM") as psumO:
            for c in range(4):
                OT = psumO.tile([P, 512], F32, space="PSUM", tag=f"OT{c}",
                                name=f"OT{c}")
                for j in range(NST):
                    nc.tensor.matmul(OT[:], lhsT=dws[j][:],
                                     rhs=mts[j][:, 512 * c: 512 * (c + 1)],
                                     start=(j == 0), stop=(j == NST - 1))
                nc.vector.tensor_scalar_add(
                    outT_sb[:, 512 * c: 512 * (c + 1)], OT[:], baseT[:])
        nc.sync.dma_start(out=outT_d[:], in_=outT_sb[:])


def _prep_inputs(token_embeddings, dep_heads, W_comp, b_comp, w_red, b_red):
    """Host-side sharding + layout prep. One in_map per core (= per batch)."""
    token = np.ascontiguousarray(np.asarray(token_embeddings, np.float32))
    heads = np.asarray(dep_heads, np.int32)
    W = np.ascontiguousarray(np.asarray(W_comp, np.float32))
    w_ptq = np.ascontiguousarray(W.transpose(1, 0, 2).reshape(P, T * T))
    bcompT = np.ascontiguousarray(
        np.asarray(b_comp, np.float32).reshape(T, 1))
    wred = np.ascontiguousarray(
        np.asarray(w_red, np.float32).reshape(NST, P).T)
    bred = np.asarray(b_red, np.float32).reshape(1, 1)
    iota = np.arange(S, dtype=np.float16).reshape(1, S)

    in_maps = []
    for b in range(B):
        in_maps.append({
            "tokT": np.ascontiguousarray(token[b].T),
            "w_ptq": w_ptq,
            "bcompT": bcompT,
            "wred": wred,
            "heads": np.ascontiguousarray(heads[b].reshape(NST, P).T),
            "bred": bred,
            "iota": iota,
        })
    return in_maps


def kernel(**inputs):
    if "nc" not in _NC_CACHE:
        _NC_CACHE["nc"] = build_nc()
    nc = _NC_CACHE["nc"]
    in_maps = _prep_inputs(
        inputs["token_embeddings"], inputs["dep_heads"], inputs["W_comp"],
        inputs["b_comp"], inputs["w_red"], inputs["b_red"])
    res = run_bass_kernel_spmd(nc, in_maps, core_ids=list(range(N_CORES)))
    out = np.empty((B, S, T), np.float32)
    for b in range(B):
        out[b] = res.results[b]["outT"].T
    return out
